# revision 51
# baseline (speedup 1.0000x reference)
"""Self-contained Trainium2 Bass kernel for the 3-layer GIN GNN (8 NeuronCores).

kernel(**inputs) takes FULL unsharded inputs, returns FULL [256, 1] f32 output.

Design:
- Graph-aligned node sharding: 32 graphs/core, each padded to `maxg` node
  slots (multiple of 128; 256 typically) -> npad = 32*maxg slots/core.
  Static pooling boundaries at multiples of maxg.
- Global h-table rows are chunk-major: first half of each core's windows
  -> table-lo rows, second half -> table-hi rows. Each half is ONE Shared
  tensor filled by ONE AllGather, so the lo AllGather is issued mid-layer
  and overlaps the remaining windows' compute; only the hi AllGather is
  exposed at each layer boundary. The halves double as the two dma_gather
  tables (int16 index range).
- Edges bucketed per (dst 128-node tile, half), sorted by src row inside
  each bucket (HBM locality); chunk counts maxed across cores into one
  SPMD program. One gather per bucket (fine-grained buffer recycling ->
  deep gather lookahead via msg_bufs).
- Scatter-add = selection-matrix matmuls on the PE; sel matrices for all
  chunks of a bucket built in ONE DVE op (stride-0 broadcast against a
  replicated iota). The (1+eps)*h self-term is folded into the same PSUM
  accumulation as an identity-scaled matmul.
- MLPs in transposed orientation [feat(P), nodes(F)]; BN folded into
  per-partition scale/bias of ScalarE activations. Per-window batched
  own-h loads / h write-backs (one strided DMA instead of per-tile DMAs);
  per-tile transposes land in one PSUM tile moved with one strided copy.
- Pooling partials AllGathered after layer 3; final 800->1 projection
  on-device.
"""

import sys

sys.path.insert(0, "/opt/trn_rl_repo")

import numpy as np

import concourse.bass as bass  # noqa: F401
import concourse.mybir as mybir
import concourse.tile as tile
from concourse import bacc, library_config
from concourse.bass_utils import run_bass_kernel_spmd

NCORES = 8
G = 256
F_IN = 79
D = 400
BN_EPS = 1e-5

GPC = G // NCORES        # 32 graphs per core
MAXG_FLOOR = 256         # min padded nodes per graph (tests may lower)
P = 128
ELEM_H = 512             # fp16 elems per h row (1KB)
ELEM_X = 128             # fp16 elems per x row (256B)
OSL = 4                  # feature slices
SL = 100                 # slice width

# tunables for perf experiments
CFG = dict(msg_bufs=4, sl_bufs=4, zt_bufs=6, yt_bufs=5, ht_bufs=5, ot_bufs=5,
           nm_bufs=4, wk_bufs=4, swdge_queues=1, gather_rot=False,
           ag_chunks=2, pl_bufs=2,
           # timing-ablation flags (break correctness; bench only)
           skip_gather=0, skip_agg=0, skip_hwrite=0, skip_mlp=0)

F16 = mybir.dt.float16
F32 = mybir.dt.float32
I16 = mybir.dt.int16


# =================================================================== host prep
def _prep(inputs):
    x = np.asarray(inputs["x"], np.float32)
    edge_index = np.asarray(inputs["edge_index"]).astype(np.int64)
    batch = np.asarray(inputs["batch_index"]).astype(np.int64)
    n = x.shape[0]
    assert int(inputs["num_graphs"]) == G

    cnt = np.bincount(batch, minlength=G).astype(np.int64)
    gstart = np.zeros(G + 1, np.int64)
    np.cumsum(cnt, out=gstart[1:])

    maxg = max(MAXG_FLOOR, int(np.ceil(cnt.max() / P)) * P)
    win = 512 if 512 % maxg == 0 else maxg
    tpw = win // P                      # tiles per window
    gpw = win // maxg                   # graphs per window
    npad = GPC * maxg
    nrows = NCORES * npad
    split = nrows // 2
    assert npad % win == 0
    nwin = npad // win
    ntile = npad // P

    # AllGather split: global rows are chunk-major so the AllGather of shard
    # rows [q*rpc:(q+1)*rpc] lands contiguously in the per-half full tables
    # (each half is one Shared tensor written by exactly one collective, and
    # doubles as one gather table: lo = rows < split, hi = rows >= split).
    agc = 2 if CFG["ag_chunks"] > 1 and nwin % 2 == 0 else 1
    cw = nwin // agc                    # windows per AllGather chunk
    rpc = cw * win                      # rows per (core, chunk)

    g_of = batch
    rank = np.arange(n, dtype=np.int64) - gstart[g_of]
    core_of = g_of // GPC
    slot = (g_of % GPC) * maxg + rank
    row_of = (slot // rpc) * (NCORES * rpc) + core_of * rpc + (slot % rpc)

    src = edge_index[0]
    dst = edge_index[1]
    e_core = core_of[dst]
    e_tile = slot[dst] // P
    e_dloc = slot[dst] % P
    e_srow = row_of[src]
    e_hi = (e_srow >= split).astype(np.int64)

    key = e_core * (ntile * 2) + e_tile * 2 + e_hi
    sidx_all = np.where(e_hi == 1, e_srow - split, e_srow)
    order = np.lexsort((sidx_all, key))   # per-bucket ascending src rows
    skey = key[order]
    sidx = sidx_all[order]
    sdl = e_dloc[order]

    counts = np.bincount(key, minlength=NCORES * ntile * 2).reshape(
        NCORES, ntile, 2)
    cpt = np.maximum(np.ceil(counts / P).astype(np.int64).max(axis=0), 1)
    cpt_lo = [int(v) for v in cpt[:, 0]]
    cpt_hi = [int(v) for v in cpt[:, 1]]

    CLO = [sum(cpt_lo[w * tpw:(w + 1) * tpw]) for w in range(nwin)]
    CHI = [sum(cpt_hi[w * tpw:(w + 1) * tpw]) for w in range(nwin)]
    totch = sum(CLO) + sum(CHI)

    # chunk base per (window, half, tile-in-window), matching device layout
    ch_base = np.zeros((nwin, 2, tpw), np.int64)
    off = 0
    for w in range(nwin):
        for t in range(tpw):
            ch_base[w, 0, t] = off
            off += cpt_lo[w * tpw + t]
        for t in range(tpw):
            ch_base[w, 1, t] = off
            off += cpt_hi[w * tpw + t]
    assert off == totch

    idx_all = np.zeros((NCORES, totch * P), np.int16)
    dst_all = np.full((NCORES, totch * P), -1.0, np.float16)
    gcnt = np.ones((NCORES, ntile * 2), np.int32)   # valid idxs per bucket

    bstart = np.searchsorted(skey, np.arange(NCORES * ntile * 2))
    bend = np.append(bstart[1:], len(skey))
    bstart = bstart.reshape(NCORES, ntile, 2)
    bend = bend.reshape(NCORES, ntile, 2)

    for c in range(NCORES):
        for w in range(nwin):
            for half in (0, 1):
                for t in range(tpw):
                    gt = w * tpw + t
                    b0, b1 = bstart[c, gt, half], bend[c, gt, half]
                    ne = b1 - b0
                    base = int(ch_base[w, half, t]) * P
                    idx_all[c, base:base + ne] = sidx[b0:b1].astype(np.int16)
                    dst_all[c, base:base + ne] = sdl[b0:b1].astype(np.float16)
                    gcnt[c, gt * 2 + half] = max(int(ne), 1)

    S_tot = totch * 8
    iw = idx_all.reshape(NCORES, totch * 8, 16).transpose(0, 2, 1)
    idx_wrapped = np.tile(iw, (1, 8, 1))                        # [C, 128, S]
    dw = dst_all.reshape(NCORES, totch, P).transpose(0, 2, 1)   # [C, 128, totch]

    nmax = int(max(max(cpt_lo), max(cpt_hi)))
    iota_rep = np.tile(np.arange(P, dtype=np.float16)[None, :], (P, nmax))

    x_nm = np.zeros((nrows, ELEM_X), np.float16)
    x_nm[row_of, :F_IN] = x.astype(np.float16)
    x_sl = np.zeros((NCORES, npad, P), np.float16)              # slot-ordered
    x_sl[core_of, slot] = x_nm[row_of, :P]

    real = np.zeros((NCORES, npad), np.float32)
    real[core_of, slot] = 1.0
    maskneg = (1.0 - real) * -60000.0

    w = {k: np.asarray(v, np.float32) for k, v in inputs.items()
         if k not in ("x", "edge_index", "batch_index", "num_graphs")}
    s1 = w["mlp1_bn_g"] / np.sqrt(w["mlp1_bn_v"] + BN_EPS)
    t1 = (w["mlp1_b1"] - w["mlp1_bn_m"]) * s1 + w["mlp1_bn_b"]
    s2 = w["mlp2_bn_g"] / np.sqrt(w["mlp2_bn_v"] + BN_EPS)
    t2 = (w["mlp2_b1"] - w["mlp2_bn_m"]) * s2 + w["mlp2_bn_b"]

    w1p = np.zeros((80, D), np.float16)
    w1p[:F_IN] = w["mlp1_w1"].astype(np.float16)

    def ksl(mat):       # [400, 400] -> [100(ki), 4(ko), 400(out)]
        return np.ascontiguousarray(
            mat.astype(np.float16).reshape(4, SL, D).transpose(1, 0, 2))

    def sb4(vec):       # [400] -> [100, 4]
        return np.ascontiguousarray(vec.astype(np.float32).reshape(4, SL).T)

    meta = dict(
        maxg=maxg, win=win, tpw=tpw, gpw=gpw, npad=npad, nrows=nrows,
        split=split, nwin=nwin, ntile=ntile, agc=agc, cw=cw, rpc=rpc,
        cpt_lo=cpt_lo, cpt_hi=cpt_hi, CLO=CLO, CHI=CHI,
        totch=totch, S_tot=S_tot,
        eps1=float(1.0 + np.asarray(inputs["eps1"], np.float32)[0]),
        eps2=float(1.0 + np.asarray(inputs["eps2"], np.float32)[0]),
        eps3=float(1.0 + np.asarray(inputs["eps3"], np.float32)[0]),
        out_b=float(w["out_b"][0]),
    )

    meta["nmax"] = nmax

    shared = {
        "x_nm": x_nm, "w1p": w1p, "iota_rep": iota_rep,
        "m1s": sb4(s1), "m1t": sb4(t1),
        "m1w2": ksl(w["mlp1_w2"]), "m1b2": sb4(w["mlp1_b2"]),
        "m2w1": ksl(w["mlp2_w1"]),
        "m2s": sb4(s2), "m2t": sb4(t2),
        "m2w2": ksl(w["mlp2_w2"]), "m2b2": sb4(w["mlp2_b2"]),
        "ow1": ksl(w["out1_w"]), "ob1": sb4(w["out1_b"]),
        "ow2": ksl(w["out2_w"]), "ob2": sb4(w["out2_b"]),
        "ow3": ksl(w["out3_w"]), "ob3": sb4(w["out3_b"]),
        "pwmax": np.ascontiguousarray(np.broadcast_to(
            w["out_w"][:D, 0].astype(np.float32)[None, :], (P, D))),
        "pwmean": np.ascontiguousarray(np.broadcast_to(
            w["out_w"][D:, 0].astype(np.float32)[None, :], (P, D))),
        "invcnt": (1.0 / np.maximum(cnt, 1)).astype(np.float32)[:, None],
    }
    in_maps = []
    for c in range(NCORES):
        m = dict(shared)
        m["xT"] = np.ascontiguousarray(x_sl[c].T)
        m["idxs"] = np.ascontiguousarray(idx_wrapped[c])
        m["gcnt"] = np.ascontiguousarray(gcnt[c][None, :])
        m["dstf"] = np.ascontiguousarray(dw[c])
        m["maskneg"] = np.ascontiguousarray(np.broadcast_to(
            maskneg[c][None, :], (SL, npad))).astype(np.float16)
        m["maskmul"] = np.ascontiguousarray(np.broadcast_to(
            real[c][None, :], (SL, npad))).astype(np.float16)
        in_maps.append(m)
    return meta, in_maps


# =================================================================== device IR
def _build(meta, skip_coll=False, only_layer=None, pool_mode="full", reps=1):
    import contextlib

    nwin, nrows, split = meta["nwin"], meta["nrows"], meta["split"]
    npad, win, tpw, gpw = meta["npad"], meta["win"], meta["tpw"], meta["gpw"]
    maxg, cw, rpc = meta["maxg"], meta["cw"], meta["rpc"]
    agc = meta["agc"]
    cpt_lo, cpt_hi = meta["cpt_lo"], meta["cpt_hi"]
    CLO, CHI, S_tot, totch = meta["CLO"], meta["CHI"], meta["S_tot"], meta["totch"]
    CLmax = max(cpt_lo)
    CHmax = max(cpt_hi)

    nc = bacc.Bacc("TRN2", target_bir_lowering=False, debug=False,
                   num_devices=NCORES, num_swdge_queues=CFG["swdge_queues"])

    nmax = meta["nmax"]
    din = {}
    for name, shape, dt in [
        ("x_nm", [nrows, ELEM_X], F16), ("xT", [P, npad], F16),
        ("iota_rep", [P, nmax * P], F16),
        ("idxs", [P, S_tot], I16), ("dstf", [P, totch], F16),
        ("gcnt", [1, meta["ntile"] * 2], mybir.dt.int32),
        ("w1p", [80, D], F16), ("m1s", [SL, 4], F32), ("m1t", [SL, 4], F32),
        ("m1w2", [SL, 4, D], F16), ("m1b2", [SL, 4], F32),
        ("m2w1", [SL, 4, D], F16), ("m2s", [SL, 4], F32), ("m2t", [SL, 4], F32),
        ("m2w2", [SL, 4, D], F16), ("m2b2", [SL, 4], F32),
        ("ow1", [SL, 4, D], F16), ("ob1", [SL, 4], F32),
        ("ow2", [SL, 4, D], F16), ("ob2", [SL, 4], F32),
        ("ow3", [SL, 4, D], F16), ("ob3", [SL, 4], F32),
        ("pwmax", [P, D], F32), ("pwmean", [P, D], F32),
        ("invcnt", [G, 1], F32),
        ("maskneg", [SL, npad], F16), ("maskmul", [SL, npad], F16),
    ]:
        din[name] = nc.dram_tensor(name, shape, dt, kind="ExternalInput")
    out_t = nc.dram_tensor("out", [G, 1], F32, kind="ExternalOutput")

    eq = mybir.AluOpType.is_equal
    AF = mybir.ActivationFunctionType

    # chunk bases, same layout as host
    ch_base = []
    off = 0
    for w in range(nwin):
        lo_b = []
        for t in range(tpw):
            lo_b.append(off)
            off += cpt_lo[w * tpw + t]
        hi_b = []
        for t in range(tpw):
            hi_b.append(off)
            off += cpt_hi[w * tpw + t]
        ch_base.append((lo_b, hi_b))

    with tile.TileContext(nc) as tc:
        nc.gpsimd.load_library(library_config.mlp)
        with contextlib.ExitStack() as ctx:
            cst = ctx.enter_context(tc.tile_pool(name="cst", bufs=1))
            dram = ctx.enter_context(tc.tile_pool(name="drm", bufs=1, space="DRAM"))
            p_lo = ctx.enter_context(tc.tile_pool(name="p_lo", bufs=CFG["msg_bufs"]))
            p_hi = ctx.enter_context(tc.tile_pool(name="p_hi", bufs=CFG["msg_bufs"]))
            p_wk = ctx.enter_context(tc.tile_pool(name="p_wk", bufs=CFG["wk_bufs"]))
            p_sl = ctx.enter_context(tc.tile_pool(name="p_sl", bufs=CFG["sl_bufs"]))
            p_zt = ctx.enter_context(tc.tile_pool(name="p_zt", bufs=CFG["zt_bufs"]))
            p_yt = ctx.enter_context(tc.tile_pool(name="p_yt", bufs=CFG["yt_bufs"]))
            p_ht = ctx.enter_context(tc.tile_pool(name="p_ht", bufs=CFG["ht_bufs"]))
            p_ot = ctx.enter_context(tc.tile_pool(name="p_ot", bufs=CFG["ot_bufs"]))
            p_nm = ctx.enter_context(tc.tile_pool(name="p_nm", bufs=CFG["nm_bufs"]))
            p_pl = ctx.enter_context(tc.tile_pool(name="p_pl", bufs=CFG["pl_bufs"]))
            ps_agg = ctx.enter_context(tc.tile_pool(name="ps_agg", bufs=2, space="PSUM"))
            ps_tr = ctx.enter_context(tc.tile_pool(name="ps_tr", bufs=2, space="PSUM"))
            ps_mm = ctx.enter_context(tc.tile_pool(name="ps_mm", bufs=4, space="PSUM"))

            # resident constants
            sb = {}
            for name in din:
                if name in ("x_nm", "xT", "invcnt", "maskneg", "maskmul"):
                    continue
                t = cst.tile(list(din[name].shape), din[name].dtype,
                             name=f"sb_{name}")
                full = tuple(slice(None) for _ in din[name].shape)
                nc.sync.dma_start(t[full], din[name][full])
                sb[name] = t

            iota_r = cst.tile([P, P], F16, name="iota_r")
            nc.gpsimd.iota(iota_r[:], pattern=[[1, P]], base=0,
                           channel_multiplier=0,
                           allow_small_or_imprecise_dtypes=True)
            pcol = cst.tile([P, 1], F32, name="pcol")
            nc.gpsimd.iota(pcol[:], pattern=[[1, 1]], base=0,
                           channel_multiplier=1,
                           allow_small_or_imprecise_dtypes=True)
            ident = cst.tile([P, P], F16, name="ident")
            nc.vector.tensor_scalar(ident[:], iota_r[:], pcol[:, :1], None, eq)
            identf = cst.tile([P, P], F32, name="identf")
            nc.vector.tensor_copy(identf[:], ident[:])
            # (1+eps)*I per layer: folds the self-term into the PE scatter-add
            idep = {}
            for li, ev in ((1, meta["eps1"]), (2, meta["eps2"]),
                           (3, meta["eps3"])):
                ti = cst.tile([P, P], F16, name=f"idep{li}")
                nc.scalar.mul(ti[:], ident[:], ev)
                idep[li] = ti

            acc_max = [cst.tile([SL, GPC], F32, name=f"accm{o}") for o in range(OSL)]
            acc_sum = [cst.tile([SL, GPC], F32, name=f"accs{o}") for o in range(OSL)]

            if CFG["skip_gather"]:
                m_st_lo = cst.tile([P, CLmax, ELEM_H], F16, name="m_st_lo")
                nc.vector.memset(m_st_lo[:], 0.0)
                m_st_hi = cst.tile([P, CHmax, ELEM_H], F16, name="m_st_hi")
                nc.vector.memset(m_st_hi[:], 0.0)

            def sel_block(a, n):
                """Batched sel matrices for chunk slots [a, a+n): [P, n, P]."""
                s = p_sl.tile([P, n, P], F16, tag="sel", name="selb")
                nc.vector.tensor_tensor(
                    out=s[:, :, :],
                    in0=sb["iota_rep"][:, :n * P].rearrange(
                        "p (a c) -> p a c", a=n),
                    in1=sb["dstf"][:, a:a + n].broadcast_to([P, n, P]),
                    op=eq)
                return s

            def mm4(zts, wname, kp, ksl_n, act_pool, func, scale4, bias4, dt=F16):
                """For o in 0..3: act(sum_k W[k,o]^T @ zts[k]). Returns 4 tiles.
                zts entries may be tiles or pre-sliced APs of [kp, win]."""
                outs = []
                for o in range(OSL):
                    psy = ps_mm.tile([SL, win], F32, tag="psmm", name="psy")
                    kn = 1 if CFG["skip_mlp"] else ksl_n
                    for k in range(kn):
                        lhsT = (sb[wname][:kp, k, o * SL:(o + 1) * SL]
                                if ksl_n > 1
                                else sb[wname][:kp, o * SL:(o + 1) * SL])
                        nc.tensor.matmul(psy[:, :], lhsT=lhsT,
                                         rhs=zts[k],
                                         start=(k == 0), stop=(k == kn - 1))
                    t = act_pool.tile([SL, win], dt, tag=f"a_{act_pool.name}",
                                      name="actt")
                    sc = scale4[:, o:o + 1] if scale4 is not None else 1.0
                    nc.scalar.activation(t[:], psy[:, :], func,
                                         bias=bias4[:, o:o + 1], scale=sc)
                    outs.append(t)
                return outs

            for _rep in range(reps):
             h1_sh = dram.tile([npad, ELEM_H], F16, name="h1_sh")
             h2_sh = dram.tile([npad, ELEM_H], F16, name="h2_sh")
             if agc == 2:
                 # per-half Shared tables: each written by ONE AllGather, so
                 # the lo half can collect while hi windows still compute.
                 h1_tab = [dram.tile([split, ELEM_H], F16, name=f"h1_f{q}",
                                     addr_space="Shared") for q in range(2)]
                 h2_tab = [dram.tile([split, ELEM_H], F16, name=f"h2_f{q}",
                                     addr_space="Shared") for q in range(2)]
                 h1_lo, h1_hi = h1_tab[0][:, :], h1_tab[1][:, :]
                 h2_lo, h2_hi = h2_tab[0][:, :], h2_tab[1][:, :]
             else:
                 h1_full = dram.tile([nrows, ELEM_H], F16, name="h1_full",
                                     addr_space="Shared")
                 h2_full = dram.tile([nrows, ELEM_H], F16, name="h2_full",
                                     addr_space="Shared")
                 h1_tab, h2_tab = [h1_full], [h2_full]
                 h1_lo, h1_hi = h1_full[:, :], h1_full[split:, :]
                 h2_lo, h2_hi = h2_full[:, :], h2_full[split:, :]
             pmax_in = dram.tile([GPC, D], F32, name="pmax_in")
             psm_in = dram.tile([GPC, D], F32, name="psm_in")
             pmax_all = dram.tile([G, D], F32, name="pmax_all", addr_space="Shared")
             psm_all = dram.tile([G, D], F32, name="psm_all", addr_space="Shared")
             for layer in ((1, 2, 3) if only_layer is None else only_layer):
                if layer == 1:
                    table_lo, table_hi, elem = din["x_nm"][:, :], din["x_nm"][split:, :], ELEM_X
                    eps = meta["eps1"]
                elif layer == 2:
                    table_lo, table_hi, elem = h1_lo, h1_hi, ELEM_H
                    eps = meta["eps2"]
                else:
                    table_lo, table_hi, elem = h2_lo, h2_hi, ELEM_H
                    eps = meta["eps3"]
                shard_r = None if layer == 1 else (h1_sh if layer == 2 else h2_sh)
                shard_w = h1_sh if layer == 1 else (h2_sh if layer == 2 else None)

                for w in range(nwin):
                    mtag = "g1" if layer == 1 else "g2"
                    mlo_t, mhi_t = {}, {}
                    for t in range(tpw):
                        gt = w * tpw + t
                        nl, nh = cpt_lo[gt], cpt_hi[gt]
                        lo0, hi0 = ch_base[w][0][t], ch_base[w][1][t]
                        if CFG["skip_gather"]:
                            mlo_t[t], mhi_t[t] = m_st_lo, m_st_hi
                            continue
                        mlo = p_lo.tile([P, CLmax, elem], F16, tag=f"{mtag}lo",
                                        name="mlo")
                        mhi = p_hi.tile([P, CHmax, elem], F16, tag=f"{mtag}hi",
                                        name="mhi")
                        nc.gpsimd.dma_gather(
                            mlo[:, :nl, :], table_lo,
                            sb["idxs"][:, lo0 * 8:(lo0 + nl) * 8],
                            nl * P, nl * P, elem, single_packet=False,
                            queue_num=0)
                        nc.gpsimd.dma_gather(
                            mhi[:, :nh, :], table_hi,
                            sb["idxs"][:, hi0 * 8:(hi0 + nh) * 8],
                            nh * P, nh * P, elem, single_packet=False,
                            queue_num=0)
                        mlo_t[t], mhi_t[t] = mlo, mhi

                    # ---- aggregation + transposed z
                    if layer == 1:
                        xt = p_wk.tile([80, win], F16, tag="xt", name="xt")
                        nc.sync.dma_start(xt[:], din["xT"][:80, w * win:(w + 1) * win])
                        psz = ps_agg.tile([80, win], F32, tag="agg", name="psz")
                        for t in range(tpw):
                            gt = w * tpw + t
                            nl, nh = cpt_lo[gt], cpt_hi[gt]
                            mlo, mhi = mlo_t[t], mhi_t[t]
                            lo0, hi0 = ch_base[w][0][t], ch_base[w][1][t]
                            jn = 1 if CFG["skip_agg"] else nl + nh
                            s_lo = sel_block(ch_base[w][0][t], nl)
                            s_hi = (sel_block(ch_base[w][1][t], nh)
                                    if jn > nl else None)
                            for j in range(jn):
                                if j < nl:
                                    slot = ch_base[w][0][t] + j
                                    rhs = mlo[:, slot - lo0, :80]
                                    s = s_lo[:, j, :]
                                else:
                                    slot = ch_base[w][1][t] + (j - nl)
                                    rhs = mhi[:, slot - hi0, :80]
                                    s = s_hi[:, j - nl, :]
                                nc.tensor.matmul(
                                    psz[:, t * P:(t + 1) * P], lhsT=rhs, rhs=s,
                                    start=(j == 0), stop=False)
                            nc.tensor.matmul(
                                psz[:, t * P:(t + 1) * P],
                                lhsT=idep[1][:80, :80],
                                rhs=xt[:, t * P:(t + 1) * P],
                                start=False, stop=True)
                        z1 = p_zt.tile([80, win], F16, tag="zt1", name="z1")
                        nc.any.tensor_copy(out=z1[:], in_=psz[:, :])
                        yt = mm4([z1[:80, :]], "w1p", 80, 1, p_yt, AF.Relu,
                                 sb["m1s"], sb["m1t"])
                        w2n, b2n, own, obn = "m1w2", "m1b2", "ow1", "ob1"
                    else:
                        zt_all = p_zt.tile([SL, 4, win], F16, tag="zt2",
                                           name="zt_all")
                        hwin = p_wk.tile([P, tpw, D], F16, tag="hwin",
                                         name="hwin")
                        nc.sync.dma_start(
                            hwin[:, :, :],
                            shard_r[w * win:(w + 1) * win, :D].rearrange(
                                "(t p) d -> p t d", p=P))
                        for t in range(tpw):
                            gt = w * tpw + t
                            nl, nh = cpt_lo[gt], cpt_hi[gt]
                            mlo, mhi = mlo_t[t], mhi_t[t]
                            lo0, hi0 = ch_base[w][0][t], ch_base[w][1][t]
                            psa = ps_agg.tile([P, D], F32, tag="agg", name="psa")
                            jn = 1 if CFG["skip_agg"] else nl + nh
                            s_lo = sel_block(ch_base[w][0][t], nl)
                            s_hi = (sel_block(ch_base[w][1][t], nh)
                                    if jn > nl else None)
                            for j in range(jn):
                                if j < nl:
                                    slot = ch_base[w][0][t] + j
                                    rhs = mlo[:, slot - lo0, :D]
                                    s = s_lo[:, j, :]
                                else:
                                    slot = ch_base[w][1][t] + (j - nl)
                                    rhs = mhi[:, slot - hi0, :D]
                                    s = s_hi[:, j - nl, :]
                                nc.tensor.matmul(
                                    psa[:, :], lhsT=s, rhs=rhs,
                                    start=(j == 0), stop=False)
                            nc.tensor.matmul(
                                psa[:, :], lhsT=idep[layer][:],
                                rhs=hwin[:, t, :], start=False, stop=True)
                            znm = p_nm.tile([P, D], F16, tag="znm", name="znm")
                            nc.any.tensor_copy(out=znm[:], in_=psa[:, :])
                            pst = ps_tr.tile([SL, 4 * P], F16, tag="tr",
                                             name="pst")
                            for fs in range(4):
                                nc.tensor.transpose(
                                    pst[:, fs * P:(fs + 1) * P],
                                    znm[:, fs * SL:(fs + 1) * SL], ident[:])
                            nc.any.tensor_copy(
                                out=zt_all[:, :, t * P:(t + 1) * P],
                                in_=pst[:, :].rearrange("p (f c) -> p f c", f=4))
                        ztiles = [zt_all[:, k, :] for k in range(4)]
                        yt = mm4(ztiles, "m2w1", SL, 4, p_yt, AF.Relu,
                                 sb["m2s"], sb["m2t"])
                        if layer == 2:
                            w2n, b2n, own, obn = "m2w2", "m2b2", "ow2", "ob2"
                        else:
                            w2n, b2n, own, obn = "m2w2", "m2b2", "ow3", "ob3"

                    ht = mm4([y[:SL, :] for y in yt], w2n, SL, 4, p_ht,
                             AF.Relu, None, sb[b2n])
                    hot = mm4([h[:SL, :] for h in ht], own, SL, 4, p_ot,
                              AF.Tanh, None, sb[obn],
                              dt=(F32 if layer == 3 else F16))

                    if layer < 3:
                        if not CFG["skip_hwrite"]:
                            hwout = p_nm.tile([P, tpw, D], F16, tag="hnm",
                                              name="hwout")
                            for t in range(tpw):
                                ps2 = ps_tr.tile([P, D], F16, tag="tr",
                                                 name="ps2")
                                for fs in range(4):
                                    nc.tensor.transpose(
                                        ps2[:, fs * SL:(fs + 1) * SL],
                                        hot[fs][:, t * P:(t + 1) * P],
                                        ident[:SL, :SL])
                                nc.any.tensor_copy(out=hwout[:, t, :],
                                                   in_=ps2[:, :])
                            nc.sync.dma_start(
                                shard_w[w * win:(w + 1) * win, :D].rearrange(
                                    "(t p) d -> p t d", p=P),
                                hwout[:, :, :])
                    elif pool_mode != "none":
                        mneg_t = p_pl.tile([SL, win], F16, tag="mneg", name="mneg")
                        nc.sync.dma_start(mneg_t[:], din["maskneg"][:, w * win:(w + 1) * win])
                        mmul_t = p_pl.tile([SL, win], F16, tag="mmul", name="mmul")
                        nc.sync.dma_start(mmul_t[:], din["maskmul"][:, w * win:(w + 1) * win])
                        mneg = mneg_t[:, :]
                        mmul = mmul_t[:, :]
                        for o in range(OSL):
                            hm = p_pl.tile([SL, win], F32, tag="hm", name="hm")
                            nc.vector.tensor_tensor(
                                out=hm[:], in0=hot[o][:], in1=mneg,
                                op=mybir.AluOpType.add)
                            hs2 = p_pl.tile([SL, win], F32, tag="hs2", name="hs2")
                            nc.vector.tensor_tensor(
                                out=hs2[:], in0=hot[o][:], in1=mmul,
                                op=mybir.AluOpType.mult)
                            for gg in range(gpw):
                                gl = w * gpw + gg
                                nc.vector.tensor_reduce(
                                    out=acc_max[o][:, gl:gl + 1],
                                    in_=hm[:, gg * maxg:(gg + 1) * maxg],
                                    axis=mybir.AxisListType.X,
                                    op=mybir.AluOpType.max)
                                nc.vector.tensor_reduce(
                                    out=acc_sum[o][:, gl:gl + 1],
                                    in_=hs2[:, gg * maxg:(gg + 1) * maxg],
                                    axis=mybir.AxisListType.X,
                                    op=mybir.AluOpType.add)

                    if (layer < 3 and not skip_coll
                            and (w + 1) % cw == 0):
                        q = (w + 1) // cw - 1
                        tabs = h1_tab if layer == 1 else h2_tab
                        nc.gpsimd.collective_compute(
                            "AllGather", mybir.AluOpType.bypass,
                            replica_groups=[list(range(NCORES))],
                            ins=[shard_w[q * rpc:(q + 1) * rpc, :].opt()],
                            outs=[tabs[q].opt()])

             # pooling finalize
             do_pool = (only_layer is None or 3 in only_layer) and pool_mode == "full"
             for acc, bounce, allb in (((acc_max, pmax_in, pmax_all),
                                       (acc_sum, psm_in, psm_all)) if do_pool else ()):
                asm = p_pl.tile([GPC, D], F32, tag="asm", name="asm")
                for o in range(OSL):
                    ps3 = ps_tr.tile([GPC, SL], F32, tag="tr", name="ps3")
                    nc.tensor.transpose(ps3[:, :], acc[o][:, :], identf[:SL, :SL])
                    nc.any.tensor_copy(out=asm[:, o * SL:(o + 1) * SL], in_=ps3[:, :])
                nc.sync.dma_start(bounce[:, :], asm[:])
                if not skip_coll:
                    nc.gpsimd.collective_compute(
                        "AllGather", mybir.AluOpType.bypass,
                        replica_groups=[list(range(NCORES))],
                        ins=[bounce.opt()], outs=[allb.opt()])

            for g0 in range(G // P if do_pool else 0):
                mx = p_pl.tile([P, D], F32, tag="mx", name="mx")
                sm = p_pl.tile([P, D], F32, tag="sm", name="sm")
                nc.sync.dma_start(mx[:], pmax_all[g0 * P:(g0 + 1) * P, :])
                nc.sync.dma_start(sm[:], psm_all[g0 * P:(g0 + 1) * P, :])
                ic = p_pl.tile([P, 1], F32, tag="ic", name="ic")
                nc.sync.dma_start(ic[:], din["invcnt"][g0 * P:(g0 + 1) * P, :])
                t1 = p_pl.tile([P, D], F32, tag="t1", name="t1")
                nc.vector.tensor_tensor(
                    out=t1[:], in0=mx[:],
                    in1=sb["pwmax"][:, :],
                    op=mybir.AluOpType.mult)
                d1 = p_pl.tile([P, 1], F32, tag="d1", name="d1")
                nc.vector.tensor_reduce(out=d1[:], in_=t1[:],
                                        axis=mybir.AxisListType.X,
                                        op=mybir.AluOpType.add)
                t2 = p_pl.tile([P, D], F32, tag="t2", name="t2")
                nc.vector.tensor_tensor(
                    out=t2[:], in0=sm[:],
                    in1=sb["pwmean"][:, :],
                    op=mybir.AluOpType.mult)
                d2 = p_pl.tile([P, 1], F32, tag="d2", name="d2")
                nc.vector.tensor_reduce(out=d2[:], in_=t2[:],
                                        axis=mybir.AxisListType.X,
                                        op=mybir.AluOpType.add)
                nc.vector.tensor_tensor(out=d2[:], in0=d2[:], in1=ic[:],
                                        op=mybir.AluOpType.mult)
                nc.vector.tensor_add(out=d1[:], in0=d1[:], in1=d2[:])
                nc.vector.tensor_scalar_add(d1[:], d1[:], meta["out_b"])
                nc.sync.dma_start(out_t[g0 * P:(g0 + 1) * P, :], d1[:])

    nc.finalize()
    return nc


_CACHE = {}


def build_all(inputs):
    """Returns (nc, meta, in_maps); caches the compiled program."""
    meta, in_maps = _prep(inputs)
    key = (meta["nwin"], meta["totch"], tuple(meta["cpt_lo"]),
           tuple(meta["cpt_hi"]), meta["eps1"], meta["eps2"], meta["eps3"],
           meta["out_b"], meta["agc"])
    if key not in _CACHE:
        _CACHE.clear()
        _CACHE[key] = _build(meta)
    return _CACHE[key], meta, in_maps


def kernel(**inputs):
    nc, meta, in_maps = build_all(inputs)
    res = run_bass_kernel_spmd(nc, in_maps, core_ids=list(range(NCORES)))
    return np.asarray(res.results[0]["out"], np.float32)



# revision 52
# speedup vs baseline: 1.0081x; 1.0081x over previous
"""Self-contained Trainium2 Bass kernel for the 3-layer GIN GNN (8 NeuronCores).

kernel(**inputs) takes FULL unsharded inputs, returns FULL [256, 1] f32 output.

Design:
- Graph-aligned node sharding: 32 graphs/core, each padded to `maxg` node
  slots (multiple of 128; 256 typically) -> npad = 32*maxg slots/core.
  Static pooling boundaries at multiples of maxg.
- Global h-table rows are chunk-major: first half of each core's windows
  -> table-lo rows, second half -> table-hi rows. Each half is ONE Shared
  tensor filled by ONE AllGather, so the lo AllGather is issued mid-layer
  and overlaps the remaining windows' compute; only the hi AllGather is
  exposed at each layer boundary. The halves double as the two dma_gather
  tables (int16 index range).
- Edges bucketed per (dst 128-node tile, half), sorted by src row inside
  each bucket (HBM locality); chunk counts maxed across cores into one
  SPMD program. One gather per bucket (fine-grained buffer recycling ->
  deep gather lookahead via msg_bufs).
- Scatter-add = selection-matrix matmuls on the PE; sel matrices for all
  chunks of a bucket built in ONE DVE op (stride-0 broadcast against a
  replicated iota). The (1+eps)*h self-term is folded into the same PSUM
  accumulation as an identity-scaled matmul.
- MLPs in transposed orientation [feat(P), nodes(F)]; BN folded into
  per-partition scale/bias of ScalarE activations. Per-window batched
  own-h loads / h write-backs (one strided DMA instead of per-tile DMAs);
  per-tile transposes land in one PSUM tile moved with one strided copy.
- Pooling partials AllGathered after layer 3; final 800->1 projection
  on-device.
"""

import sys

sys.path.insert(0, "/opt/trn_rl_repo")

import numpy as np

import concourse.bass as bass  # noqa: F401
import concourse.mybir as mybir
import concourse.tile as tile
from concourse import bacc, library_config
from concourse.bass_utils import run_bass_kernel_spmd

NCORES = 8
G = 256
F_IN = 79
D = 400
BN_EPS = 1e-5

GPC = G // NCORES        # 32 graphs per core
MAXG_FLOOR = 256         # min padded nodes per graph (tests may lower)
P = 128
ELEM_H = 512             # fp16 elems per h row (1KB)
ELEM_X = 128             # fp16 elems per x row (256B)
OSL = 4                  # feature slices
SL = 100                 # slice width

# tunables for perf experiments
CFG = dict(msg_bufs=5, sl_bufs=4, zt_bufs=4, yt_bufs=5, ht_bufs=5, ot_bufs=5,
           nm_bufs=4, wk_bufs=4, swdge_queues=1, gather_rot=False,
           ag_chunks=2, pl_bufs=2,
           # timing-ablation flags (break correctness; bench only)
           skip_gather=0, skip_agg=0, skip_hwrite=0, skip_mlp=0)

F16 = mybir.dt.float16
F32 = mybir.dt.float32
I16 = mybir.dt.int16


# =================================================================== host prep
def _prep(inputs):
    x = np.asarray(inputs["x"], np.float32)
    edge_index = np.asarray(inputs["edge_index"]).astype(np.int64)
    batch = np.asarray(inputs["batch_index"]).astype(np.int64)
    n = x.shape[0]
    assert int(inputs["num_graphs"]) == G

    cnt = np.bincount(batch, minlength=G).astype(np.int64)
    gstart = np.zeros(G + 1, np.int64)
    np.cumsum(cnt, out=gstart[1:])

    maxg = max(MAXG_FLOOR, int(np.ceil(cnt.max() / P)) * P)
    win = 512 if 512 % maxg == 0 else maxg
    tpw = win // P                      # tiles per window
    gpw = win // maxg                   # graphs per window
    npad = GPC * maxg
    nrows = NCORES * npad
    split = nrows // 2
    assert npad % win == 0
    nwin = npad // win
    ntile = npad // P

    # AllGather split: global rows are chunk-major so the AllGather of shard
    # rows [q*rpc:(q+1)*rpc] lands contiguously in the per-half full tables
    # (each half is one Shared tensor written by exactly one collective, and
    # doubles as one gather table: lo = rows < split, hi = rows >= split).
    agc = 2 if CFG["ag_chunks"] > 1 and nwin % 2 == 0 else 1
    cw = nwin // agc                    # windows per AllGather chunk
    rpc = cw * win                      # rows per (core, chunk)

    g_of = batch
    rank = np.arange(n, dtype=np.int64) - gstart[g_of]
    core_of = g_of // GPC
    slot = (g_of % GPC) * maxg + rank
    row_of = (slot // rpc) * (NCORES * rpc) + core_of * rpc + (slot % rpc)

    src = edge_index[0]
    dst = edge_index[1]
    e_core = core_of[dst]
    e_tile = slot[dst] // P
    e_dloc = slot[dst] % P
    e_srow = row_of[src]
    e_hi = (e_srow >= split).astype(np.int64)

    key = e_core * (ntile * 2) + e_tile * 2 + e_hi
    sidx_all = np.where(e_hi == 1, e_srow - split, e_srow)
    order = np.lexsort((sidx_all, key))   # per-bucket ascending src rows
    skey = key[order]
    sidx = sidx_all[order]
    sdl = e_dloc[order]

    counts = np.bincount(key, minlength=NCORES * ntile * 2).reshape(
        NCORES, ntile, 2)
    cpt = np.maximum(np.ceil(counts / P).astype(np.int64).max(axis=0), 1)
    cpt_lo = [int(v) for v in cpt[:, 0]]
    cpt_hi = [int(v) for v in cpt[:, 1]]

    CLO = [sum(cpt_lo[w * tpw:(w + 1) * tpw]) for w in range(nwin)]
    CHI = [sum(cpt_hi[w * tpw:(w + 1) * tpw]) for w in range(nwin)]
    totch = sum(CLO) + sum(CHI)

    # chunk base per (window, half, tile-in-window), matching device layout
    ch_base = np.zeros((nwin, 2, tpw), np.int64)
    off = 0
    for w in range(nwin):
        for t in range(tpw):
            ch_base[w, 0, t] = off
            off += cpt_lo[w * tpw + t]
        for t in range(tpw):
            ch_base[w, 1, t] = off
            off += cpt_hi[w * tpw + t]
    assert off == totch

    idx_all = np.zeros((NCORES, totch * P), np.int16)
    dst_all = np.full((NCORES, totch * P), -1.0, np.float16)
    gcnt = np.ones((NCORES, ntile * 2), np.int32)   # valid idxs per bucket

    bstart = np.searchsorted(skey, np.arange(NCORES * ntile * 2))
    bend = np.append(bstart[1:], len(skey))
    bstart = bstart.reshape(NCORES, ntile, 2)
    bend = bend.reshape(NCORES, ntile, 2)

    for c in range(NCORES):
        for w in range(nwin):
            for half in (0, 1):
                for t in range(tpw):
                    gt = w * tpw + t
                    b0, b1 = bstart[c, gt, half], bend[c, gt, half]
                    ne = b1 - b0
                    base = int(ch_base[w, half, t]) * P
                    idx_all[c, base:base + ne] = sidx[b0:b1].astype(np.int16)
                    dst_all[c, base:base + ne] = sdl[b0:b1].astype(np.float16)
                    gcnt[c, gt * 2 + half] = max(int(ne), 1)

    S_tot = totch * 8
    iw = idx_all.reshape(NCORES, totch * 8, 16).transpose(0, 2, 1)
    idx_wrapped = np.tile(iw, (1, 8, 1))                        # [C, 128, S]
    dw = dst_all.reshape(NCORES, totch, P).transpose(0, 2, 1)   # [C, 128, totch]

    nmax = int(max(max(cpt_lo), max(cpt_hi)))
    iota_rep = np.tile(np.arange(P, dtype=np.float16)[None, :], (P, nmax))

    x_nm = np.zeros((nrows, ELEM_X), np.float16)
    x_nm[row_of, :F_IN] = x.astype(np.float16)
    x_sl = np.zeros((NCORES, npad, P), np.float16)              # slot-ordered
    x_sl[core_of, slot] = x_nm[row_of, :P]

    real = np.zeros((NCORES, npad), np.float32)
    real[core_of, slot] = 1.0
    maskneg = (1.0 - real) * -60000.0

    w = {k: np.asarray(v, np.float32) for k, v in inputs.items()
         if k not in ("x", "edge_index", "batch_index", "num_graphs")}
    s1 = w["mlp1_bn_g"] / np.sqrt(w["mlp1_bn_v"] + BN_EPS)
    t1 = (w["mlp1_b1"] - w["mlp1_bn_m"]) * s1 + w["mlp1_bn_b"]
    s2 = w["mlp2_bn_g"] / np.sqrt(w["mlp2_bn_v"] + BN_EPS)
    t2 = (w["mlp2_b1"] - w["mlp2_bn_m"]) * s2 + w["mlp2_bn_b"]

    w1p = np.zeros((80, D), np.float16)
    w1p[:F_IN] = w["mlp1_w1"].astype(np.float16)

    def ksl(mat):       # [400, 400] -> [100(ki), 4(ko), 400(out)]
        return np.ascontiguousarray(
            mat.astype(np.float16).reshape(4, SL, D).transpose(1, 0, 2))

    def sb4(vec):       # [400] -> [100, 4]
        return np.ascontiguousarray(vec.astype(np.float32).reshape(4, SL).T)

    meta = dict(
        maxg=maxg, win=win, tpw=tpw, gpw=gpw, npad=npad, nrows=nrows,
        split=split, nwin=nwin, ntile=ntile, agc=agc, cw=cw, rpc=rpc,
        cpt_lo=cpt_lo, cpt_hi=cpt_hi, CLO=CLO, CHI=CHI,
        totch=totch, S_tot=S_tot,
        eps1=float(1.0 + np.asarray(inputs["eps1"], np.float32)[0]),
        eps2=float(1.0 + np.asarray(inputs["eps2"], np.float32)[0]),
        eps3=float(1.0 + np.asarray(inputs["eps3"], np.float32)[0]),
        out_b=float(w["out_b"][0]),
    )

    meta["nmax"] = nmax

    shared = {
        "x_nm": x_nm, "w1p": w1p, "iota_rep": iota_rep,
        "m1s": sb4(s1), "m1t": sb4(t1),
        "m1w2": ksl(w["mlp1_w2"]), "m1b2": sb4(w["mlp1_b2"]),
        "m2w1": ksl(w["mlp2_w1"]),
        "m2s": sb4(s2), "m2t": sb4(t2),
        "m2w2": ksl(w["mlp2_w2"]), "m2b2": sb4(w["mlp2_b2"]),
        "ow1": ksl(w["out1_w"]), "ob1": sb4(w["out1_b"]),
        "ow2": ksl(w["out2_w"]), "ob2": sb4(w["out2_b"]),
        "ow3": ksl(w["out3_w"]), "ob3": sb4(w["out3_b"]),
        "pwmax": np.ascontiguousarray(np.broadcast_to(
            w["out_w"][:D, 0].astype(np.float32)[None, :], (P, D))),
        "pwmean": np.ascontiguousarray(np.broadcast_to(
            w["out_w"][D:, 0].astype(np.float32)[None, :], (P, D))),
        "invcnt": (1.0 / np.maximum(cnt, 1)).astype(np.float32)[:, None],
    }
    in_maps = []
    for c in range(NCORES):
        m = dict(shared)
        m["xT"] = np.ascontiguousarray(x_sl[c].T)
        m["idxs"] = np.ascontiguousarray(idx_wrapped[c])
        m["gcnt"] = np.ascontiguousarray(gcnt[c][None, :])
        m["dstf"] = np.ascontiguousarray(dw[c])
        m["maskneg"] = np.ascontiguousarray(np.broadcast_to(
            maskneg[c][None, :], (SL, npad))).astype(np.float16)
        m["maskmul"] = np.ascontiguousarray(np.broadcast_to(
            real[c][None, :], (SL, npad))).astype(np.float16)
        in_maps.append(m)
    return meta, in_maps


# =================================================================== device IR
def _build(meta, skip_coll=False, only_layer=None, pool_mode="full", reps=1):
    import contextlib

    nwin, nrows, split = meta["nwin"], meta["nrows"], meta["split"]
    npad, win, tpw, gpw = meta["npad"], meta["win"], meta["tpw"], meta["gpw"]
    maxg, cw, rpc = meta["maxg"], meta["cw"], meta["rpc"]
    agc = meta["agc"]
    cpt_lo, cpt_hi = meta["cpt_lo"], meta["cpt_hi"]
    CLO, CHI, S_tot, totch = meta["CLO"], meta["CHI"], meta["S_tot"], meta["totch"]
    CLmax = max(cpt_lo)
    CHmax = max(cpt_hi)

    nc = bacc.Bacc("TRN2", target_bir_lowering=False, debug=False,
                   num_devices=NCORES, num_swdge_queues=CFG["swdge_queues"])

    nmax = meta["nmax"]
    din = {}
    for name, shape, dt in [
        ("x_nm", [nrows, ELEM_X], F16), ("xT", [P, npad], F16),
        ("iota_rep", [P, nmax * P], F16),
        ("idxs", [P, S_tot], I16), ("dstf", [P, totch], F16),
        ("gcnt", [1, meta["ntile"] * 2], mybir.dt.int32),
        ("w1p", [80, D], F16), ("m1s", [SL, 4], F32), ("m1t", [SL, 4], F32),
        ("m1w2", [SL, 4, D], F16), ("m1b2", [SL, 4], F32),
        ("m2w1", [SL, 4, D], F16), ("m2s", [SL, 4], F32), ("m2t", [SL, 4], F32),
        ("m2w2", [SL, 4, D], F16), ("m2b2", [SL, 4], F32),
        ("ow1", [SL, 4, D], F16), ("ob1", [SL, 4], F32),
        ("ow2", [SL, 4, D], F16), ("ob2", [SL, 4], F32),
        ("ow3", [SL, 4, D], F16), ("ob3", [SL, 4], F32),
        ("pwmax", [P, D], F32), ("pwmean", [P, D], F32),
        ("invcnt", [G, 1], F32),
        ("maskneg", [SL, npad], F16), ("maskmul", [SL, npad], F16),
    ]:
        din[name] = nc.dram_tensor(name, shape, dt, kind="ExternalInput")
    out_t = nc.dram_tensor("out", [G, 1], F32, kind="ExternalOutput")

    eq = mybir.AluOpType.is_equal
    AF = mybir.ActivationFunctionType

    # chunk bases, same layout as host
    ch_base = []
    off = 0
    for w in range(nwin):
        lo_b = []
        for t in range(tpw):
            lo_b.append(off)
            off += cpt_lo[w * tpw + t]
        hi_b = []
        for t in range(tpw):
            hi_b.append(off)
            off += cpt_hi[w * tpw + t]
        ch_base.append((lo_b, hi_b))

    with tile.TileContext(nc) as tc:
        nc.gpsimd.load_library(library_config.mlp)
        with contextlib.ExitStack() as ctx:
            cst = ctx.enter_context(tc.tile_pool(name="cst", bufs=1))
            dram = ctx.enter_context(tc.tile_pool(name="drm", bufs=1, space="DRAM"))
            p_lo = ctx.enter_context(tc.tile_pool(name="p_lo", bufs=CFG["msg_bufs"]))
            p_hi = ctx.enter_context(tc.tile_pool(name="p_hi", bufs=CFG["msg_bufs"]))
            p_wk = ctx.enter_context(tc.tile_pool(name="p_wk", bufs=CFG["wk_bufs"]))
            p_sl = ctx.enter_context(tc.tile_pool(name="p_sl", bufs=CFG["sl_bufs"]))
            p_zt = ctx.enter_context(tc.tile_pool(name="p_zt", bufs=CFG["zt_bufs"]))
            p_yt = ctx.enter_context(tc.tile_pool(name="p_yt", bufs=CFG["yt_bufs"]))
            p_ht = ctx.enter_context(tc.tile_pool(name="p_ht", bufs=CFG["ht_bufs"]))
            p_ot = ctx.enter_context(tc.tile_pool(name="p_ot", bufs=CFG["ot_bufs"]))
            p_nm = ctx.enter_context(tc.tile_pool(name="p_nm", bufs=CFG["nm_bufs"]))
            p_pl = ctx.enter_context(tc.tile_pool(name="p_pl", bufs=CFG["pl_bufs"]))
            ps_agg = ctx.enter_context(tc.tile_pool(name="ps_agg", bufs=2, space="PSUM"))
            ps_tr = ctx.enter_context(tc.tile_pool(name="ps_tr", bufs=2, space="PSUM"))
            ps_mm = ctx.enter_context(tc.tile_pool(name="ps_mm", bufs=4, space="PSUM"))

            # resident constants
            sb = {}
            for name in din:
                if name in ("x_nm", "xT", "invcnt", "maskneg", "maskmul"):
                    continue
                t = cst.tile(list(din[name].shape), din[name].dtype,
                             name=f"sb_{name}")
                full = tuple(slice(None) for _ in din[name].shape)
                nc.sync.dma_start(t[full], din[name][full])
                sb[name] = t

            iota_r = cst.tile([P, P], F16, name="iota_r")
            nc.gpsimd.iota(iota_r[:], pattern=[[1, P]], base=0,
                           channel_multiplier=0,
                           allow_small_or_imprecise_dtypes=True)
            pcol = cst.tile([P, 1], F32, name="pcol")
            nc.gpsimd.iota(pcol[:], pattern=[[1, 1]], base=0,
                           channel_multiplier=1,
                           allow_small_or_imprecise_dtypes=True)
            ident = cst.tile([P, P], F16, name="ident")
            nc.vector.tensor_scalar(ident[:], iota_r[:], pcol[:, :1], None, eq)
            identf = cst.tile([P, P], F32, name="identf")
            nc.vector.tensor_copy(identf[:], ident[:])
            # (1+eps)*I per layer: folds the self-term into the PE scatter-add
            idep = {}
            for li, ev in ((1, meta["eps1"]), (2, meta["eps2"]),
                           (3, meta["eps3"])):
                ti = cst.tile([P, P], F16, name=f"idep{li}")
                nc.scalar.mul(ti[:], ident[:], ev)
                idep[li] = ti

            acc_max = [cst.tile([SL, GPC], F32, name=f"accm{o}") for o in range(OSL)]
            acc_sum = [cst.tile([SL, GPC], F32, name=f"accs{o}") for o in range(OSL)]

            if CFG["skip_gather"]:
                m_st_lo = cst.tile([P, CLmax, ELEM_H], F16, name="m_st_lo")
                nc.vector.memset(m_st_lo[:], 0.0)
                m_st_hi = cst.tile([P, CHmax, ELEM_H], F16, name="m_st_hi")
                nc.vector.memset(m_st_hi[:], 0.0)

            def sel_block(a, n):
                """Batched sel matrices for chunk slots [a, a+n): [P, n, P]."""
                s = p_sl.tile([P, n, P], F16, tag="sel", name="selb")
                nc.vector.tensor_tensor(
                    out=s[:, :, :],
                    in0=sb["iota_rep"][:, :n * P].rearrange(
                        "p (a c) -> p a c", a=n),
                    in1=sb["dstf"][:, a:a + n].broadcast_to([P, n, P]),
                    op=eq)
                return s

            def mm4(zts, wname, kp, ksl_n, act_pool, func, scale4, bias4, dt=F16):
                """For o in 0..3: act(sum_k W[k,o]^T @ zts[k]). Returns 4 tiles.
                zts entries may be tiles or pre-sliced APs of [kp, win]."""
                outs = []
                for o in range(OSL):
                    psy = ps_mm.tile([SL, win], F32, tag="psmm", name="psy")
                    kn = 1 if CFG["skip_mlp"] else ksl_n
                    for k in range(kn):
                        lhsT = (sb[wname][:kp, k, o * SL:(o + 1) * SL]
                                if ksl_n > 1
                                else sb[wname][:kp, o * SL:(o + 1) * SL])
                        nc.tensor.matmul(psy[:, :], lhsT=lhsT,
                                         rhs=zts[k],
                                         start=(k == 0), stop=(k == kn - 1))
                    t = act_pool.tile([SL, win], dt, tag=f"a_{act_pool.name}",
                                      name="actt")
                    sc = scale4[:, o:o + 1] if scale4 is not None else 1.0
                    nc.scalar.activation(t[:], psy[:, :], func,
                                         bias=bias4[:, o:o + 1], scale=sc)
                    outs.append(t)
                return outs

            for _rep in range(reps):
             h1_sh = dram.tile([npad, ELEM_H], F16, name="h1_sh")
             h2_sh = dram.tile([npad, ELEM_H], F16, name="h2_sh")
             if agc == 2:
                 # per-half Shared tables: each written by ONE AllGather, so
                 # the lo half can collect while hi windows still compute.
                 h1_tab = [dram.tile([split, ELEM_H], F16, name=f"h1_f{q}",
                                     addr_space="Shared") for q in range(2)]
                 h2_tab = [dram.tile([split, ELEM_H], F16, name=f"h2_f{q}",
                                     addr_space="Shared") for q in range(2)]
                 h1_lo, h1_hi = h1_tab[0][:, :], h1_tab[1][:, :]
                 h2_lo, h2_hi = h2_tab[0][:, :], h2_tab[1][:, :]
             else:
                 h1_full = dram.tile([nrows, ELEM_H], F16, name="h1_full",
                                     addr_space="Shared")
                 h2_full = dram.tile([nrows, ELEM_H], F16, name="h2_full",
                                     addr_space="Shared")
                 h1_tab, h2_tab = [h1_full], [h2_full]
                 h1_lo, h1_hi = h1_full[:, :], h1_full[split:, :]
                 h2_lo, h2_hi = h2_full[:, :], h2_full[split:, :]
             pmax_in = dram.tile([GPC, D], F32, name="pmax_in")
             psm_in = dram.tile([GPC, D], F32, name="psm_in")
             pmax_all = dram.tile([G, D], F32, name="pmax_all", addr_space="Shared")
             psm_all = dram.tile([G, D], F32, name="psm_all", addr_space="Shared")
             for layer in ((1, 2, 3) if only_layer is None else only_layer):
                if layer == 1:
                    table_lo, table_hi, elem = din["x_nm"][:, :], din["x_nm"][split:, :], ELEM_X
                    eps = meta["eps1"]
                elif layer == 2:
                    table_lo, table_hi, elem = h1_lo, h1_hi, ELEM_H
                    eps = meta["eps2"]
                else:
                    table_lo, table_hi, elem = h2_lo, h2_hi, ELEM_H
                    eps = meta["eps3"]
                shard_r = None if layer == 1 else (h1_sh if layer == 2 else h2_sh)
                shard_w = h1_sh if layer == 1 else (h2_sh if layer == 2 else None)

                for w in range(nwin):
                    mtag = "g1" if layer == 1 else "g2"
                    mlo_t, mhi_t = {}, {}
                    for t in range(tpw):
                        gt = w * tpw + t
                        nl, nh = cpt_lo[gt], cpt_hi[gt]
                        lo0, hi0 = ch_base[w][0][t], ch_base[w][1][t]
                        if CFG["skip_gather"]:
                            mlo_t[t], mhi_t[t] = m_st_lo, m_st_hi
                            continue
                        mlo = p_lo.tile([P, CLmax, elem], F16, tag=f"{mtag}lo",
                                        name="mlo")
                        mhi = p_hi.tile([P, CHmax, elem], F16, tag=f"{mtag}hi",
                                        name="mhi")
                        nc.gpsimd.dma_gather(
                            mlo[:, :nl, :], table_lo,
                            sb["idxs"][:, lo0 * 8:(lo0 + nl) * 8],
                            nl * P, nl * P, elem, single_packet=False,
                            queue_num=0)
                        nc.gpsimd.dma_gather(
                            mhi[:, :nh, :], table_hi,
                            sb["idxs"][:, hi0 * 8:(hi0 + nh) * 8],
                            nh * P, nh * P, elem, single_packet=False,
                            queue_num=0)
                        mlo_t[t], mhi_t[t] = mlo, mhi

                    # ---- aggregation + transposed z
                    if layer == 1:
                        xt = p_wk.tile([80, win], F16, tag="xt", name="xt")
                        nc.sync.dma_start(xt[:], din["xT"][:80, w * win:(w + 1) * win])
                        psz = ps_agg.tile([80, win], F32, tag="agg", name="psz")
                        for t in range(tpw):
                            gt = w * tpw + t
                            nl, nh = cpt_lo[gt], cpt_hi[gt]
                            mlo, mhi = mlo_t[t], mhi_t[t]
                            lo0, hi0 = ch_base[w][0][t], ch_base[w][1][t]
                            jn = 1 if CFG["skip_agg"] else nl + nh
                            s_lo = sel_block(ch_base[w][0][t], nl)
                            s_hi = (sel_block(ch_base[w][1][t], nh)
                                    if jn > nl else None)
                            for j in range(jn):
                                if j < nl:
                                    slot = ch_base[w][0][t] + j
                                    rhs = mlo[:, slot - lo0, :80]
                                    s = s_lo[:, j, :]
                                else:
                                    slot = ch_base[w][1][t] + (j - nl)
                                    rhs = mhi[:, slot - hi0, :80]
                                    s = s_hi[:, j - nl, :]
                                nc.tensor.matmul(
                                    psz[:, t * P:(t + 1) * P], lhsT=rhs, rhs=s,
                                    start=(j == 0), stop=False)
                            nc.tensor.matmul(
                                psz[:, t * P:(t + 1) * P],
                                lhsT=idep[1][:80, :80],
                                rhs=xt[:, t * P:(t + 1) * P],
                                start=False, stop=True)
                        z1 = p_zt.tile([80, win], F16, tag="zt1", name="z1")
                        nc.any.tensor_copy(out=z1[:], in_=psz[:, :])
                        yt = mm4([z1[:80, :]], "w1p", 80, 1, p_yt, AF.Relu,
                                 sb["m1s"], sb["m1t"])
                        w2n, b2n, own, obn = "m1w2", "m1b2", "ow1", "ob1"
                    else:
                        zt_all = p_zt.tile([SL, 4, win], F16, tag="zt2",
                                           name="zt_all")
                        hwin = p_wk.tile([P, tpw, D], F16, tag="hwin",
                                         name="hwin")
                        nc.sync.dma_start(
                            hwin[:, :, :],
                            shard_r[w * win:(w + 1) * win, :D].rearrange(
                                "(t p) d -> p t d", p=P))
                        for t in range(tpw):
                            gt = w * tpw + t
                            nl, nh = cpt_lo[gt], cpt_hi[gt]
                            mlo, mhi = mlo_t[t], mhi_t[t]
                            lo0, hi0 = ch_base[w][0][t], ch_base[w][1][t]
                            psa = ps_agg.tile([P, D], F32, tag="agg", name="psa")
                            jn = 1 if CFG["skip_agg"] else nl + nh
                            s_lo = sel_block(ch_base[w][0][t], nl)
                            s_hi = (sel_block(ch_base[w][1][t], nh)
                                    if jn > nl else None)
                            for j in range(jn):
                                if j < nl:
                                    slot = ch_base[w][0][t] + j
                                    rhs = mlo[:, slot - lo0, :D]
                                    s = s_lo[:, j, :]
                                else:
                                    slot = ch_base[w][1][t] + (j - nl)
                                    rhs = mhi[:, slot - hi0, :D]
                                    s = s_hi[:, j - nl, :]
                                nc.tensor.matmul(
                                    psa[:, :], lhsT=s, rhs=rhs,
                                    start=(j == 0), stop=False)
                            nc.tensor.matmul(
                                psa[:, :], lhsT=idep[layer][:],
                                rhs=hwin[:, t, :], start=False, stop=True)
                            znm = p_nm.tile([P, D], F16, tag="znm", name="znm")
                            nc.any.tensor_copy(out=znm[:], in_=psa[:, :])
                            pst = ps_tr.tile([SL, 4 * P], F16, tag="tr",
                                             name="pst")
                            for fs in range(4):
                                nc.tensor.transpose(
                                    pst[:, fs * P:(fs + 1) * P],
                                    znm[:, fs * SL:(fs + 1) * SL], ident[:])
                            nc.any.tensor_copy(
                                out=zt_all[:, :, t * P:(t + 1) * P],
                                in_=pst[:, :].rearrange("p (f c) -> p f c", f=4))
                        ztiles = [zt_all[:, k, :] for k in range(4)]
                        yt = mm4(ztiles, "m2w1", SL, 4, p_yt, AF.Relu,
                                 sb["m2s"], sb["m2t"])
                        if layer == 2:
                            w2n, b2n, own, obn = "m2w2", "m2b2", "ow2", "ob2"
                        else:
                            w2n, b2n, own, obn = "m2w2", "m2b2", "ow3", "ob3"

                    ht = mm4([y[:SL, :] for y in yt], w2n, SL, 4, p_ht,
                             AF.Relu, None, sb[b2n])
                    hot = mm4([h[:SL, :] for h in ht], own, SL, 4, p_ot,
                              AF.Tanh, None, sb[obn],
                              dt=(F32 if layer == 3 else F16))

                    if layer < 3:
                        if not CFG["skip_hwrite"]:
                            hwout = p_nm.tile([P, tpw, D], F16, tag="hnm",
                                              name="hwout")
                            for t in range(tpw):
                                ps2 = ps_tr.tile([P, D], F16, tag="tr",
                                                 name="ps2")
                                for fs in range(4):
                                    nc.tensor.transpose(
                                        ps2[:, fs * SL:(fs + 1) * SL],
                                        hot[fs][:, t * P:(t + 1) * P],
                                        ident[:SL, :SL])
                                nc.any.tensor_copy(out=hwout[:, t, :],
                                                   in_=ps2[:, :])
                            nc.sync.dma_start(
                                shard_w[w * win:(w + 1) * win, :D].rearrange(
                                    "(t p) d -> p t d", p=P),
                                hwout[:, :, :])
                    elif pool_mode != "none":
                        mneg_t = p_pl.tile([SL, win], F16, tag="mneg", name="mneg")
                        nc.sync.dma_start(mneg_t[:], din["maskneg"][:, w * win:(w + 1) * win])
                        mmul_t = p_pl.tile([SL, win], F16, tag="mmul", name="mmul")
                        nc.sync.dma_start(mmul_t[:], din["maskmul"][:, w * win:(w + 1) * win])
                        mneg = mneg_t[:, :]
                        mmul = mmul_t[:, :]
                        for o in range(OSL):
                            hm = p_pl.tile([SL, win], F32, tag="hm", name="hm")
                            nc.vector.tensor_tensor(
                                out=hm[:], in0=hot[o][:], in1=mneg,
                                op=mybir.AluOpType.add)
                            hs2 = p_pl.tile([SL, win], F32, tag="hs2", name="hs2")
                            nc.vector.tensor_tensor(
                                out=hs2[:], in0=hot[o][:], in1=mmul,
                                op=mybir.AluOpType.mult)
                            for gg in range(gpw):
                                gl = w * gpw + gg
                                nc.vector.tensor_reduce(
                                    out=acc_max[o][:, gl:gl + 1],
                                    in_=hm[:, gg * maxg:(gg + 1) * maxg],
                                    axis=mybir.AxisListType.X,
                                    op=mybir.AluOpType.max)
                                nc.vector.tensor_reduce(
                                    out=acc_sum[o][:, gl:gl + 1],
                                    in_=hs2[:, gg * maxg:(gg + 1) * maxg],
                                    axis=mybir.AxisListType.X,
                                    op=mybir.AluOpType.add)

                    if (layer < 3 and not skip_coll
                            and (w + 1) % cw == 0):
                        q = (w + 1) // cw - 1
                        tabs = h1_tab if layer == 1 else h2_tab
                        nc.gpsimd.collective_compute(
                            "AllGather", mybir.AluOpType.bypass,
                            replica_groups=[list(range(NCORES))],
                            ins=[shard_w[q * rpc:(q + 1) * rpc, :].opt()],
                            outs=[tabs[q].opt()])

             # pooling finalize
             do_pool = (only_layer is None or 3 in only_layer) and pool_mode == "full"
             for acc, bounce, allb in (((acc_max, pmax_in, pmax_all),
                                       (acc_sum, psm_in, psm_all)) if do_pool else ()):
                asm = p_pl.tile([GPC, D], F32, tag="asm", name="asm")
                for o in range(OSL):
                    ps3 = ps_tr.tile([GPC, SL], F32, tag="tr", name="ps3")
                    nc.tensor.transpose(ps3[:, :], acc[o][:, :], identf[:SL, :SL])
                    nc.any.tensor_copy(out=asm[:, o * SL:(o + 1) * SL], in_=ps3[:, :])
                nc.sync.dma_start(bounce[:, :], asm[:])
                if not skip_coll:
                    nc.gpsimd.collective_compute(
                        "AllGather", mybir.AluOpType.bypass,
                        replica_groups=[list(range(NCORES))],
                        ins=[bounce.opt()], outs=[allb.opt()])

            for g0 in range(G // P if do_pool else 0):
                mx = p_pl.tile([P, D], F32, tag="mx", name="mx")
                sm = p_pl.tile([P, D], F32, tag="sm", name="sm")
                nc.sync.dma_start(mx[:], pmax_all[g0 * P:(g0 + 1) * P, :])
                nc.sync.dma_start(sm[:], psm_all[g0 * P:(g0 + 1) * P, :])
                ic = p_pl.tile([P, 1], F32, tag="ic", name="ic")
                nc.sync.dma_start(ic[:], din["invcnt"][g0 * P:(g0 + 1) * P, :])
                t1 = p_pl.tile([P, D], F32, tag="t1", name="t1")
                nc.vector.tensor_tensor(
                    out=t1[:], in0=mx[:],
                    in1=sb["pwmax"][:, :],
                    op=mybir.AluOpType.mult)
                d1 = p_pl.tile([P, 1], F32, tag="d1", name="d1")
                nc.vector.tensor_reduce(out=d1[:], in_=t1[:],
                                        axis=mybir.AxisListType.X,
                                        op=mybir.AluOpType.add)
                t2 = p_pl.tile([P, D], F32, tag="t2", name="t2")
                nc.vector.tensor_tensor(
                    out=t2[:], in0=sm[:],
                    in1=sb["pwmean"][:, :],
                    op=mybir.AluOpType.mult)
                d2 = p_pl.tile([P, 1], F32, tag="d2", name="d2")
                nc.vector.tensor_reduce(out=d2[:], in_=t2[:],
                                        axis=mybir.AxisListType.X,
                                        op=mybir.AluOpType.add)
                nc.vector.tensor_tensor(out=d2[:], in0=d2[:], in1=ic[:],
                                        op=mybir.AluOpType.mult)
                nc.vector.tensor_add(out=d1[:], in0=d1[:], in1=d2[:])
                nc.vector.tensor_scalar_add(d1[:], d1[:], meta["out_b"])
                nc.sync.dma_start(out_t[g0 * P:(g0 + 1) * P, :], d1[:])

    nc.finalize()
    return nc


_CACHE = {}


def build_all(inputs):
    """Returns (nc, meta, in_maps); caches the compiled program."""
    meta, in_maps = _prep(inputs)
    key = (meta["nwin"], meta["totch"], tuple(meta["cpt_lo"]),
           tuple(meta["cpt_hi"]), meta["eps1"], meta["eps2"], meta["eps3"],
           meta["out_b"], meta["agc"])
    if key not in _CACHE:
        _CACHE.clear()
        _CACHE[key] = _build(meta)
    return _CACHE[key], meta, in_maps


def kernel(**inputs):
    nc, meta, in_maps = build_all(inputs)
    res = run_bass_kernel_spmd(nc, in_maps, core_ids=list(range(NCORES)))
    return np.asarray(res.results[0]["out"], np.float32)

